# revision 2
# baseline (speedup 1.0000x reference)
"""Parameterized builder for the causal self-attention kernel (v3).

Per-core program (core c = batch c//2, head-group c%2, 8 heads each):
  phase 1: QKV projections via 3-term residual fp8 DoubleRow matmuls:
           x*8 = xh + xl (e4m3 hi/lo), W*64 = Wh + Wl; PSUM accumulates
           xh@Wh + xl@Wh + xh@Wl over K=256-per-instruction DoubleRow
           steps; PSUM->SBUF copies apply the 1/512 descale (ACT engine
           for Q^T/K^T -> bf16, DVE for V -> bf16 [t, 8*66] layout);
           per-head ones-columns memset for softmax denominators;
           optional exact bias adds (skipped when biases are all zero).
  phase 2: per (head, q-chunk 512): S^T = K^T.T @ Q^T bf16 (valid
           columns packed into <=1024-col PSUM groups), one exp per
           group on ACT (scale=1/8 fused, out bf16), triangular-mask
           multiplies on diagonal blocks, O^T[66, q] += V.T @ P^T bf16;
           row 64 accumulates denominators via the ones-columns;
           normalize -> Y^T.  Out-projection chunks of the previous
           q-chunk interleave at head starts to fill PE bubbles.
  phase 3: out^T = Wp_loc.T @ Y^T (bf16), PSUM->SBUF bf16, DMA out.
reps: repeat the body for slope timing.  phases: enable subsets.
"""
import numpy as np
import ml_dtypes
from contextlib import ExitStack

import concourse.bass as bass
import concourse.mybir as mybir
import concourse.tile as tile
from concourse import bacc

F32 = mybir.dt.float32
BF16 = mybir.dt.bfloat16
FP8 = mybir.dt.float8e4
EXP = mybir.ActivationFunctionType.Exp
DR = mybir.MatmulPerfMode.DoubleRow

B, T, C, H = 4, 2048, 1024, 16
D = 64
HL = 8
CL = 512
VS = D + 2          # per-head V stride: 64 v + ones + pad
VW = HL * VS        # 528
SCALE = 1.0 / 8.0
SX, SW = 8.0, 64.0  # fp8 pre-quantization scales for x and W
N_CORES = 8


def build(n_cores=N_CORES, reps=1, phases=(1, 1, 1), emit_bias=False):
    nc = bacc.Bacc("TRN2", target_bir_lowering=False, debug=False,
                   num_devices=n_cores)
    xh = nc.dram_tensor("xh", [C, T], FP8, kind="ExternalInput")
    xl = nc.dram_tensor("xl", [C, T], FP8, kind="ExternalInput")
    wqh = nc.dram_tensor("wqh", [C, CL], FP8, kind="ExternalInput")
    wql = nc.dram_tensor("wql", [C, CL], FP8, kind="ExternalInput")
    wkh = nc.dram_tensor("wkh", [C, CL], FP8, kind="ExternalInput")
    wkl = nc.dram_tensor("wkl", [C, CL], FP8, kind="ExternalInput")
    wvh = nc.dram_tensor("wvh", [C, VW], FP8, kind="ExternalInput")
    wvl = nc.dram_tensor("wvl", [C, VW], FP8, kind="ExternalInput")
    wpT = nc.dram_tensor("wpT", [CL, C], BF16, kind="ExternalInput")
    mask = nc.dram_tensor("mask", [128, 128], F32, kind="ExternalInput")
    bqk = nc.dram_tensor("bqk", [128, 8], F32, kind="ExternalInput")
    bvp = nc.dram_tensor("bvp", [1, VW], F32, kind="ExternalInput")
    out = nc.dram_tensor("out", [C, T], BF16, kind="ExternalOutput")

    with tile.TileContext(nc) as tc:
        for _ in range(reps):
            _body(tc, (xh, xl), (wqh, wql), (wkh, wkl), (wvh, wvl),
                  wpT, mask, bqk, bvp, out, phases, emit_bias)
    nc.compile()
    return nc


def _body(tc, xhl, wq, wk, wv, wpT, mask, bqk, bvp, out, phases, emit_bias):
    nc = tc.nc
    p1, p2, p3 = phases
    DS = 1.0 / (SX * SW)

    with ExitStack() as ctx:
        persist = ctx.enter_context(tc.tile_pool(name="persist", bufs=1))
        QT = persist.tile([128, 4, T], BF16)
        KT = persist.tile([128, 4, T], BF16)
        V = persist.tile([128, 16, VW], BF16)
        YT = persist.tile([128, 4, T], BF16)
        mask_f32 = persist.tile([128, 128], F32)
        nc.sync.dma_start(out=mask_f32, in_=mask.ap())
        mask_sb = persist.tile([128, 128], BF16)
        nc.vector.tensor_copy(mask_sb, mask_f32)

        if not p1 and (p2 or p3):
            for t_ in (QT, KT, YT):
                nc.vector.memset(t_[:, 0, 0:8], 1.0)
            nc.vector.memset(V[:, 0, 0:8], 1.0)
        if not p2 and p3:
            nc.vector.memset(YT[:, 0, 0:8], 1.0)

        # ---------------- phase 1: projections ----------------
        if p1:
            with ExitStack() as pctx:
                wpool = pctx.enter_context(tc.tile_pool(name="wpool", bufs=1))
                xpool = pctx.enter_context(
                    tc.tile_pool(name="xpool", bufs=18))
                qkp = pctx.enter_context(
                    tc.tile_pool(name="qkp", bufs=8, space="PSUM"))

                def load_w(pair, width):
                    tiles = []
                    for wt in pair:
                        sb = wpool.tile([128, 8, width], FP8,
                                        tag=f"w_{wt.name}",
                                        name=f"w_{wt.name}")
                        nc.sync.dma_start(
                            out=sb,
                            in_=wt.ap().rearrange("(g p) n -> p g n", p=128))
                        tiles.append(sb)
                    return tiles

                wq_sb = load_w(wq, CL)
                wk_sb = load_w(wk, CL)
                wv_sb = load_w(wv, VW)
                if emit_bias:
                    bqk_sb = wpool.tile([128, 8], F32)
                    nc.sync.dma_start(out=bqk_sb, in_=bqk.ap())
                    bvp_sb = wpool.tile([1, VW], F32)
                    nc.sync.dma_start(out=bvp_sb, in_=bvp.ap())
                    bvb = wpool.tile([128, VW], F32)
                    nc.gpsimd.partition_broadcast(bvb, bvp_sb)

                for t4 in range(4):
                    xts = {}
                    for i, xt_d in enumerate(xhl):
                        for c in range(4):
                            xt = xpool.tile([128, 2, 512], FP8, tag="x")
                            nc.sync.dma_start(
                                out=xt,
                                in_=xt_d.ap()[c * 256:(c + 1) * 256,
                                              t4 * 512:(t4 + 1) * 512]
                                .rearrange("(s p) t -> p s t", p=128))
                            xts[(i, c)] = xt
                    # (x_idx, w_idx) residual passes: hh, lh, hl
                    passes = [(0, 0), (1, 0), (0, 1)]

                    psv = [qkp.tile([128, VW // 2], F32, tag="qk",
                                    name=f"v{t4}_{i}") for i in range(8)]
                    for pi, (xi, wi) in enumerate(passes):
                        for c in range(4):
                            for ts in range(4):
                                for half in range(2):
                                    nc.tensor.matmul(
                                        psv[2 * ts + half],
                                        xts[(xi, c)][:, :,
                                                     ts * 128:(ts + 1) * 128],
                                        wv_sb[wi][
                                            :, 2 * c:2 * c + 2,
                                            half * (VW // 2):
                                            (half + 1) * (VW // 2)],
                                        start=(pi == 0 and c == 0),
                                        stop=(pi == 2 and c == 3),
                                        perf_mode=DR)
                    for ts in range(4):
                        tc16 = t4 * 4 + ts
                        for half in range(2):
                            sl = V[:, tc16,
                                   half * (VW // 2):(half + 1) * (VW // 2)]
                            bsl = psv[2 * ts + half]
                            if emit_bias:
                                nc.vector.scalar_tensor_tensor(
                                    sl, bsl, DS,
                                    bvb[:, half * (VW // 2):
                                        (half + 1) * (VW // 2)],
                                    op0=mybir.AluOpType.mult,
                                    op1=mybir.AluOpType.add)
                            else:
                                nc.vector.tensor_scalar_mul(sl, bsl, DS)

                    ps = [qkp.tile([128, 512], F32, tag="qk",
                                   name=f"qk{t4}_{i}") for i in range(8)]
                    for pi, (xi, wi) in enumerate(passes):
                        for c in range(4):
                            for m in range(4):
                                nc.tensor.matmul(
                                    ps[m],
                                    wq_sb[wi][:, 2 * c:2 * c + 2,
                                              m * 128:(m + 1) * 128],
                                    xts[(xi, c)],
                                    start=(pi == 0 and c == 0),
                                    stop=(pi == 2 and c == 3),
                                    perf_mode=DR)
                                nc.tensor.matmul(
                                    ps[4 + m],
                                    wk_sb[wi][:, 2 * c:2 * c + 2,
                                              m * 128:(m + 1) * 128],
                                    xts[(xi, c)],
                                    start=(pi == 0 and c == 0),
                                    stop=(pi == 2 and c == 3),
                                    perf_mode=DR)
                    for m in range(4):
                        qsl = QT[:, m, t4 * 512:(t4 + 1) * 512]
                        ksl = KT[:, m, t4 * 512:(t4 + 1) * 512]
                        nc.scalar.activation(
                            qsl, ps[m], mybir.ActivationFunctionType.Copy,
                            scale=DS)
                        nc.scalar.activation(
                            ksl, ps[4 + m], mybir.ActivationFunctionType.Copy,
                            scale=DS)
                        if emit_bias:
                            nc.vector.tensor_scalar_add(
                                qsl, qsl, bqk_sb[:, m:m + 1])
                            nc.vector.tensor_scalar_add(
                                ksl, ksl, bqk_sb[:, 4 + m:5 + m])

                for h in range(HL):
                    nc.vector.memset(
                        V[:, :, h * VS + D:h * VS + D + 1], 1.0)

        # ---------------- phase 2/3: attention + out-proj ----------------
        with ExitStack() as actx:
            spool = actx.enter_context(
                tc.tile_pool(name="spool", bufs=2, space="PSUM"))
            opool = actx.enter_context(
                tc.tile_pool(name="opool", bufs=2, space="PSUM"))
            ppool = actx.enter_context(tc.tile_pool(name="ppool", bufs=4))
            npool = actx.enter_context(tc.tile_pool(name="npool", bufs=3))
            outps_f = actx.enter_context(
                tc.tile_pool(name="outpsf", bufs=2, space="PSUM"))
            outpool_f = actx.enter_context(
                tc.tile_pool(name="outpoolf", bufs=3))
            wpool_f = actx.enter_context(tc.tile_pool(name="wpoolf", bufs=1))
            wp_sbf = wpool_f.tile([128, 4, C], BF16)
            if p3:
                nc.sync.dma_start(
                    out=wp_sbf,
                    in_=wpT.ap().rearrange("(j p) c -> p j c", p=128))

            def outproj_chunk(t4, cc):
                pso = outps_f.tile([128, 512], F32, tag="opsf")
                for jc in range(4):
                    nc.tensor.matmul(
                        pso, wp_sbf[:, jc, cc * 128:(cc + 1) * 128],
                        YT[:, jc, t4 * 512:(t4 + 1) * 512],
                        start=(jc == 0), stop=(jc == 3))
                oto = outpool_f.tile([128, 512], BF16, tag="otf")
                nc.vector.tensor_copy(oto, pso)
                nc.sync.dma_start(
                    out=out.ap()[cc * 128:(cc + 1) * 128,
                                 t4 * 512:(t4 + 1) * 512],
                    in_=oto)

            def make_groups(qc):
                """Pack each k-block's valid S columns [qs:512] contiguously
                into PSUM-tile groups of <=1024 columns."""
                groups, cur, used = [], [], 0
                for kb in range(4 * qc + 4):
                    qs = max(0, (kb - 4 * qc) * 128)
                    w = 512 - qs
                    if used + w > 1024:
                        groups.append(cur)
                        cur, used = [], 0
                    cur.append((kb, qs, used, w))
                    used += w
                if cur:
                    groups.append(cur)
                return groups

            def emit_s(qc, po, mg, groups, g, spt, ppt):
                s_ps = spt.tile([128, 1024], F32, tag="s")
                p_sb = ppt.tile([128, 1024], BF16, tag="p")
                for kb, qs, off, w in groups[g]:
                    nc.tensor.matmul(
                        s_ps[:, off:off + w],
                        KT[po:po + 64, mg, kb * 128:(kb + 1) * 128],
                        QT[po:po + 64, mg, qc * 512 + qs:(qc + 1) * 512],
                        start=True, stop=True)
                return s_ps, p_sb

            for qc in range(4):
                if not p2:
                    if p3:
                        for cc in range(8):
                            outproj_chunk(qc - 1 if qc > 0 else 3, cc)
                    break
                nkb = 4 * qc + 4
                groups = make_groups(qc)
                ng = len(groups)
                for h in range(HL):
                    po = (h % 2) * 64
                    mg = h // 2
                    o_ps = opool.tile([D + 2, 512], F32, tag="o")
                    tiles = [None] * ng
                    tiles[0] = emit_s(qc, po, mg, groups, 0, spool, ppool)
                    if p3 and qc > 0:
                        outproj_chunk(qc - 1, h)
                    for g in range(ng):
                        if g + 1 < ng:
                            tiles[g + 1] = emit_s(
                                qc, po, mg, groups, g + 1, spool, ppool)
                        s_ps, p_sb = tiles[g]
                        grp = groups[g]
                        gw = grp[-1][2] + grp[-1][3]
                        nc.scalar.activation(
                            p_sb[:, 0:gw], s_ps[:, 0:gw], EXP, scale=SCALE)
                        for kb, qs, off, w in grp:
                            if kb >= 4 * qc:
                                sl = p_sb[:, off:off + 128]
                                nc.vector.tensor_mul(sl, sl, mask_sb)
                        for kb, qs, off, w in grp:
                            nc.tensor.matmul(
                                o_ps[0:D + 1, qs:512],
                                V[:, kb, VS * h:VS * h + D + 1],
                                p_sb[:, off:off + w],
                                start=(kb == 0), stop=(kb == nkb - 1))
                    recip = npool.tile([1, 512], F32, tag="r")
                    nc.vector.reciprocal(recip, o_ps[D:D + 1, :])
                    bcast = npool.tile([64, 512], F32, tag="b")
                    nc.gpsimd.partition_broadcast(bcast, recip)
                    nc.vector.tensor_mul(
                        YT[po:po + 64, mg, qc * 512:(qc + 1) * 512],
                        o_ps[0:D, :], bcast)

            if p3 and p2:
                for cc in range(8):
                    outproj_chunk(3, cc)


def shard_inputs(x, Wq, bq, Wk, bk, Wv, bv, Wp, bp):
    bf16 = ml_dtypes.bfloat16
    fp8 = mybir.dt.np(FP8)
    x = np.asarray(x, dtype=np.float32)
    mask_np = np.triu(np.ones((128, 128), dtype=np.float32))

    def split8(a):
        hi = a.astype(fp8)
        lo = (a - hi.astype(np.float32)).astype(fp8)
        return hi, lo

    in_maps = []
    for c in range(N_CORES):
        b, g = c // 2, c % 2
        rows = slice(g * CL, (g + 1) * CL)

        xh, xl = split8(np.ascontiguousarray(x[b].T) * SX)

        def wsplit(W):
            return split8(
                np.ascontiguousarray(
                    np.asarray(W, dtype=np.float32)[rows].T) * SW)

        wqh, wql = wsplit(Wq)
        wkh, wkl = wsplit(Wk)

        Wl = np.asarray(Wv, dtype=np.float32)[rows]
        bl = np.asarray(bv, dtype=np.float32)[rows]
        wv_aug = np.zeros((C, VW), dtype=np.float32)
        bvp_np = np.zeros((1, VW), dtype=np.float32)
        for h in range(HL):
            wv_aug[:, h * VS:h * VS + D] = Wl[h * D:(h + 1) * D].T
            bvp_np[0, h * VS:h * VS + D] = bl[h * D:(h + 1) * D]
        wvh, wvl = split8(wv_aug * SW)
        bqk_np = np.stack(
            [np.asarray(bq, dtype=np.float32)[rows].reshape(4, 128)[m]
             for m in range(4)]
            + [np.asarray(bk, dtype=np.float32)[rows].reshape(4, 128)[m]
               for m in range(4)], axis=1)

        in_maps.append({
            "xh": xh, "xl": xl,
            "wqh": wqh, "wql": wql,
            "wkh": wkh, "wkl": wkl,
            "wvh": wvh, "wvl": wvl,
            "wpT": np.ascontiguousarray(
                np.asarray(Wp, dtype=np.float32)[:, rows].T).astype(bf16),
            "mask": mask_np,
            "bqk": bqk_np,
            "bvp": bvp_np,
        })
    return in_maps


def has_bias(bq, bk, bv):
    return any(np.any(np.asarray(b_) != 0) for b_ in (bq, bk, bv))


# ---------------- graded entry point ----------------
from concourse.bass_utils import run_bass_kernel_spmd

_NC_CACHE = {}


def kernel(x, Wq, bq, Wk, bk, Wv, bv, Wp, bp):
    eb = has_bias(bq, bk, bv)
    if eb not in _NC_CACHE:
        _NC_CACHE[eb] = build(n_cores=N_CORES, reps=1, phases=(1, 1, 1),
                              emit_bias=eb)
    nc = _NC_CACHE[eb]
    in_maps = shard_inputs(x, Wq, bq, Wk, bk, Wv, bv, Wp, bp)
    res = run_bass_kernel_spmd(nc, in_maps, core_ids=list(range(N_CORES)))
    bp32 = np.asarray(bp, dtype=np.float32)
    outs = []
    for b in range(B):
        p = (res.results[2 * b]["out"].astype(np.float32)
             + res.results[2 * b + 1]["out"].astype(np.float32))
        outs.append(p.T + bp32[None, :])
    return np.stack(outs, axis=0).astype(np.float32)
